# revision 10
# baseline (speedup 1.0000x reference)
"""KeepTopK kernel for Trainium2.

out[i, j] = x[i, j] if x[i, j] is among the top-8 of row i else 1e6.

Strategy (pure data parallel, 8 cores, 32768 rows each):
  per [128, 2048] block (1024 rows, 8 rows per partition):
    load  : whole 1MB block, issued from ACT (qActDynamicHW ring)
    DVE   : per 256-wide row segment: v8 = max8(x_seg),
            y = match_replace(x, v8, BETA)   (top-8 positions -> BETA)
    ACT   : z = -y + BETA        per half-block (0 at top-8, ~BETA else)
    POOL  : o = z + x            per half-block (exact x at top-8)
    store : per half-block (512KB), issued from SP (qSPDynamicHW ring)
Loads and stores live on different HWDGE rings so stores waiting on
compute never head-of-line-block the loads; half-block compute epilogue
lets each store start ~2us earlier than a monolithic block.
match_replace replaces exactly one occurrence per top-8 element in index
order, matching jax.lax.top_k tie semantics bitwise.
"""
import numpy as np
from contextlib import ExitStack

import concourse.bass as bass
import concourse.mybir as mybir
import concourse.tile as tile
from concourse.bass_utils import run_bass_kernel_spmd

N, E, K = 262144, 256, 8
BETA = 1000000.0
NCORES = 8
ROWS_PER_CORE = N // NCORES          # 32768
ROWS_PER_PART = 8                    # rows packed per SBUF partition
BLOCK_FREE = ROWS_PER_PART * E       # 2048
ROWS_PER_BLOCK = 128 * ROWS_PER_PART  # 1024
NBLOCKS = ROWS_PER_CORE // ROWS_PER_BLOCK  # 32
HALF = BLOCK_FREE // 2               # 1024
SEGS_PER_HALF = ROWS_PER_PART // 2   # 4
GPS_W = 768                          # add columns on GpSimd; rest on DVE

MAX_WAITS = 1


def split_sync_waits(nc, max_waits=MAX_WAITS):
    """walrus codegen rejects instructions with more than one embedded sync
    wait; hoist extras onto same-engine NoOps placed immediately before."""
    spill_id = 0
    for f in nc.m.functions:
        for bb in f.blocks:
            insts = list(bb.instructions)
            new_insts = []
            changed = False
            for inst in insts:
                si = inst.sync_info
                waits = list(si.on_wait) if si and si.on_wait else []
                if len(waits) > max_waits:
                    extra = waits[:-max_waits]
                    si.on_wait = waits[-max_waits:]
                    for j in range(0, len(extra), max_waits):
                        nop = mybir.InstNoOp(
                            name=f"waitspill-{spill_id}", ins=[], outs=[])
                        spill_id += 1
                        nop.engine = inst.engine
                        nop.sync_info = type(si)(
                            on_wait=extra[j:j + max_waits], on_update=[])
                        new_insts.append(nop)
                    changed = True
                new_insts.append(inst)
            if changed:
                bb.instructions = new_insts


def build():
    nc = bass.Bass("TRN2", target_bir_lowering=False, debug=False)
    x = nc.dram_tensor("x", [ROWS_PER_CORE, E], mybir.dt.float32,
                       kind="ExternalInput")
    out = nc.dram_tensor("out", [ROWS_PER_CORE, E], mybir.dt.float32,
                         kind="ExternalOutput")
    xap = x.ap()
    oap = out.ap()
    f32 = mybir.dt.float32
    with tile.TileContext(nc) as tc:
        with ExitStack() as ctx:
            xpool = ctx.enter_context(tc.tile_pool(name="x", bufs=5))
            ypool = ctx.enter_context(tc.tile_pool(name="y", bufs=4))
            zpool = ctx.enter_context(tc.tile_pool(name="z", bufs=4))
            opool = ctx.enter_context(tc.tile_pool(name="o", bufs=5))
            vpool = ctx.enter_context(tc.tile_pool(name="v8", bufs=4))
            xt2 = None
            for b in range(NBLOCKS):
                r0 = b * ROWS_PER_BLOCK
                dst = oap[r0:r0 + ROWS_PER_BLOCK, :].rearrange(
                    "(p r) e -> p (r e)", p=128)
                if b % 2 == 0:
                    # partition p holds rows [r0+8p : r0+8p+8] of block b
                    # (c=0) then the same rows of block b+1 (c=1)
                    src2 = xap[r0:r0 + 2 * ROWS_PER_BLOCK, :].rearrange(
                        "(c p r) e -> p c (r e)", c=2, p=128)
                    xt2 = xpool.tile([128, 2 * BLOCK_FREE], f32)
                    nc.gpsimd.dma_start(xt2[:], src2)
                boff = (b % 2) * BLOCK_FREE
                yt = ypool.tile([128, BLOCK_FREE], f32)
                v8 = vpool.tile([128, 8 * ROWS_PER_PART], f32)
                for s in range(ROWS_PER_PART):
                    seg = slice(boff + s * E, boff + (s + 1) * E)
                    nc.vector.max(v8[:, s * 8:(s + 1) * 8], xt2[:, seg])
                for s in range(ROWS_PER_PART):
                    seg = slice(boff + s * E, boff + (s + 1) * E)
                    nc.vector.match_replace(
                        yt[:, s * E:(s + 1) * E], v8[:, s * 8:(s + 1) * 8],
                        xt2[:, seg], BETA)
                zt = zpool.tile([128, BLOCK_FREE], f32)
                nc.scalar.activation(zt[:], yt[:],
                                     mybir.ActivationFunctionType.Copy,
                                     bias=BETA, scale=-1.0)
                ot = opool.tile([128, BLOCK_FREE], f32)
                nc.gpsimd.tensor_tensor(
                    ot[:], zt[:], xt2[:, boff:boff + BLOCK_FREE],
                    op=mybir.AluOpType.add)
                nc.sync.dma_start(dst, ot[:])
    split_sync_waits(nc)
    return nc


_nc_cache = None


def _get_nc():
    global _nc_cache
    if _nc_cache is None:
        _nc_cache = build()
    return _nc_cache


def kernel(x: np.ndarray, _trace: bool = False, **_trace_kwargs):
    x = np.ascontiguousarray(np.asarray(x, dtype=np.float32))
    assert x.shape == (N, E), x.shape
    nc = _get_nc()
    in_maps = [
        {"x": x[c * ROWS_PER_CORE:(c + 1) * ROWS_PER_CORE]}
        for c in range(NCORES)
    ]
    res = run_bass_kernel_spmd(nc, in_maps, core_ids=list(range(NCORES)),
                               trace=_trace, **_trace_kwargs)
    out = np.concatenate([res.results[c]["out"] for c in range(NCORES)],
                         axis=0)
    if _trace:
        return out, res
    return out


# revision 13
# speedup vs baseline: 1.0535x; 1.0535x over previous
"""KeepTopK kernel for Trainium2.

out[i, j] = x[i, j] if x[i, j] is among the top-8 of row i else 1e6.

Strategy (pure data parallel, 8 cores, 32768 rows each):
  per [128, 2048] block (1024 rows, 8 rows per partition):
    load  : whole 1MB block, issued from ACT (qActDynamicHW ring)
    DVE   : per 256-wide row segment: v8 = max8(x_seg),
            y = match_replace(x, v8, BETA)   (top-8 positions -> BETA)
    ACT   : z = -y + BETA        per half-block (0 at top-8, ~BETA else)
    POOL  : o = z + x            per half-block (exact x at top-8)
    store : per half-block (512KB), issued from SP (qSPDynamicHW ring)
Loads and stores live on different HWDGE rings so stores waiting on
compute never head-of-line-block the loads; half-block compute epilogue
lets each store start ~2us earlier than a monolithic block.
match_replace replaces exactly one occurrence per top-8 element in index
order, matching jax.lax.top_k tie semantics bitwise.
"""
import numpy as np
from contextlib import ExitStack

import concourse.bass as bass
import concourse.mybir as mybir
import concourse.tile as tile
from concourse.bass_utils import run_bass_kernel_spmd

N, E, K = 262144, 256, 8
BETA = 1000000.0
NCORES = 8
ROWS_PER_CORE = N // NCORES          # 32768
ROWS_PER_PART = 8                    # rows packed per SBUF partition
BLOCK_FREE = ROWS_PER_PART * E       # 2048
ROWS_PER_BLOCK = 128 * ROWS_PER_PART  # 1024
NBLOCKS = ROWS_PER_CORE // ROWS_PER_BLOCK  # 32
HALF = BLOCK_FREE // 2               # 1024
SEGS_PER_HALF = ROWS_PER_PART // 2   # 4
GPS_W = 768                          # add columns on GpSimd; rest on DVE

MAX_WAITS = 1


def split_sync_waits(nc, max_waits=MAX_WAITS):
    """walrus codegen rejects instructions with more than one embedded sync
    wait; hoist extras onto same-engine NoOps placed immediately before."""
    spill_id = 0
    for f in nc.m.functions:
        for bb in f.blocks:
            insts = list(bb.instructions)
            new_insts = []
            changed = False
            for inst in insts:
                si = inst.sync_info
                waits = list(si.on_wait) if si and si.on_wait else []
                if len(waits) > max_waits:
                    extra = waits[:-max_waits]
                    si.on_wait = waits[-max_waits:]
                    for j in range(0, len(extra), max_waits):
                        nop = mybir.InstNoOp(
                            name=f"waitspill-{spill_id}", ins=[], outs=[])
                        spill_id += 1
                        nop.engine = inst.engine
                        nop.sync_info = type(si)(
                            on_wait=extra[j:j + max_waits], on_update=[])
                        new_insts.append(nop)
                    changed = True
                new_insts.append(inst)
            if changed:
                bb.instructions = new_insts


def build():
    nc = bass.Bass("TRN2", target_bir_lowering=False, debug=False)
    x = nc.dram_tensor("x", [ROWS_PER_CORE, E], mybir.dt.float32,
                       kind="ExternalInput")
    out = nc.dram_tensor("out", [ROWS_PER_CORE, E], mybir.dt.float32,
                         kind="ExternalOutput")
    xap = x.ap()
    oap = out.ap()
    f32 = mybir.dt.float32
    with tile.TileContext(nc) as tc:
        with ExitStack() as ctx:
            xpool = ctx.enter_context(tc.tile_pool(name="x", bufs=5))
            ypool = ctx.enter_context(tc.tile_pool(name="y", bufs=4))
            zpool = ctx.enter_context(tc.tile_pool(name="z", bufs=4))
            opool = ctx.enter_context(tc.tile_pool(name="o", bufs=5))
            vpool = ctx.enter_context(tc.tile_pool(name="v8", bufs=4))
            xt2 = None
            for b in range(NBLOCKS):
                r0 = b * ROWS_PER_BLOCK
                dst = oap[r0:r0 + ROWS_PER_BLOCK, :].rearrange(
                    "(p r) e -> p (r e)", p=128)
                src = xap[r0:r0 + ROWS_PER_BLOCK, :].rearrange(
                    "(p r) e -> p (r e)", p=128)
                split_load = b < 2 or b >= NBLOCKS - 2
                if split_load:
                    if b % 2 == 0:
                        xt2 = xpool.tile([128, 2 * BLOCK_FREE], f32,
                                         tag="x2")
                    nc.gpsimd.dma_start(
                        xt2[:, (b % 2) * BLOCK_FREE:
                            (b % 2 + 1) * BLOCK_FREE], src)
                elif b % 2 == 0:
                    # partition p holds rows [r0+8p : r0+8p+8] of block b
                    # (c=0) then the same rows of block b+1 (c=1)
                    src2 = xap[r0:r0 + 2 * ROWS_PER_BLOCK, :].rearrange(
                        "(c p r) e -> p c (r e)", c=2, p=128)
                    xt2 = xpool.tile([128, 2 * BLOCK_FREE], f32, tag="x2")
                    nc.gpsimd.dma_start(xt2[:], src2)
                boff = (b % 2) * BLOCK_FREE
                yt = ypool.tile([128, BLOCK_FREE], f32)
                v8 = vpool.tile([128, 8 * ROWS_PER_PART], f32)
                zt = zpool.tile([128, BLOCK_FREE], f32)
                for h in range(2):
                    hs = h * SEGS_PER_HALF
                    for s in range(hs, hs + SEGS_PER_HALF):
                        seg = slice(boff + s * E, boff + (s + 1) * E)
                        nc.vector.max(v8[:, s * 8:(s + 1) * 8], xt2[:, seg])
                    for s in range(hs, hs + SEGS_PER_HALF):
                        seg = slice(boff + s * E, boff + (s + 1) * E)
                        nc.vector.match_replace(
                            yt[:, s * E:(s + 1) * E],
                            v8[:, s * 8:(s + 1) * 8], xt2[:, seg], BETA)
                    nc.scalar.activation(
                        zt[:, h * HALF:(h + 1) * HALF],
                        yt[:, h * HALF:(h + 1) * HALF],
                        mybir.ActivationFunctionType.Copy,
                        bias=BETA, scale=-1.0)
                ot = opool.tile([128, BLOCK_FREE], f32)
                nc.gpsimd.tensor_tensor(
                    ot[:], zt[:], xt2[:, boff:boff + BLOCK_FREE],
                    op=mybir.AluOpType.add)
                nc.sync.dma_start(dst, ot[:])
    split_sync_waits(nc)
    return nc


_nc_cache = None


def _get_nc():
    global _nc_cache
    if _nc_cache is None:
        _nc_cache = build()
    return _nc_cache


def kernel(x: np.ndarray, _trace: bool = False, **_trace_kwargs):
    x = np.ascontiguousarray(np.asarray(x, dtype=np.float32))
    assert x.shape == (N, E), x.shape
    nc = _get_nc()
    in_maps = [
        {"x": x[c * ROWS_PER_CORE:(c + 1) * ROWS_PER_CORE]}
        for c in range(NCORES)
    ]
    res = run_bass_kernel_spmd(nc, in_maps, core_ids=list(range(NCORES)),
                               trace=_trace, **_trace_kwargs)
    out = np.concatenate([res.results[c]["out"] for c in range(NCORES)],
                         axis=0)
    if _trace:
        return out, res
    return out


# revision 14
# speedup vs baseline: 1.0563x; 1.0027x over previous
"""KeepTopK kernel for Trainium2.

out[i, j] = x[i, j] if x[i, j] is among the top-8 of row i else 1e6.

Strategy (pure data parallel, 8 cores, 32768 rows each):
  per [128, 2048] block (1024 rows, 8 rows per partition):
    load  : whole 1MB block, issued from ACT (qActDynamicHW ring)
    DVE   : per 256-wide row segment: v8 = max8(x_seg),
            y = match_replace(x, v8, BETA)   (top-8 positions -> BETA)
    ACT   : z = -y + BETA        per half-block (0 at top-8, ~BETA else)
    POOL  : o = z + x            per half-block (exact x at top-8)
    store : per half-block (512KB), issued from SP (qSPDynamicHW ring)
Loads and stores live on different HWDGE rings so stores waiting on
compute never head-of-line-block the loads; half-block compute epilogue
lets each store start ~2us earlier than a monolithic block.
match_replace replaces exactly one occurrence per top-8 element in index
order, matching jax.lax.top_k tie semantics bitwise.
"""
import numpy as np
from contextlib import ExitStack

import concourse.bass as bass
import concourse.mybir as mybir
import concourse.tile as tile
from concourse.bass_utils import run_bass_kernel_spmd

N, E, K = 262144, 256, 8
BETA = 1000000.0
NCORES = 8
ROWS_PER_CORE = N // NCORES          # 32768
ROWS_PER_PART = 8                    # rows packed per SBUF partition
BLOCK_FREE = ROWS_PER_PART * E       # 2048
ROWS_PER_BLOCK = 128 * ROWS_PER_PART  # 1024
NBLOCKS = ROWS_PER_CORE // ROWS_PER_BLOCK  # 32
HALF = BLOCK_FREE // 2               # 1024
SEGS_PER_HALF = ROWS_PER_PART // 2   # 4
GPS_W = 768                          # add columns on GpSimd; rest on DVE

MAX_WAITS = 1


def split_sync_waits(nc, max_waits=MAX_WAITS):
    """walrus codegen rejects instructions with more than one embedded sync
    wait; hoist extras onto same-engine NoOps placed immediately before."""
    spill_id = 0
    for f in nc.m.functions:
        for bb in f.blocks:
            insts = list(bb.instructions)
            new_insts = []
            changed = False
            for inst in insts:
                si = inst.sync_info
                waits = list(si.on_wait) if si and si.on_wait else []
                if len(waits) > max_waits:
                    extra = waits[:-max_waits]
                    si.on_wait = waits[-max_waits:]
                    for j in range(0, len(extra), max_waits):
                        nop = mybir.InstNoOp(
                            name=f"waitspill-{spill_id}", ins=[], outs=[])
                        spill_id += 1
                        nop.engine = inst.engine
                        nop.sync_info = type(si)(
                            on_wait=extra[j:j + max_waits], on_update=[])
                        new_insts.append(nop)
                    changed = True
                new_insts.append(inst)
            if changed:
                bb.instructions = new_insts


def build():
    nc = bass.Bass("TRN2", target_bir_lowering=False, debug=False)
    x = nc.dram_tensor("x", [ROWS_PER_CORE, E], mybir.dt.float32,
                       kind="ExternalInput")
    out = nc.dram_tensor("out", [ROWS_PER_CORE, E], mybir.dt.float32,
                         kind="ExternalOutput")
    xap = x.ap()
    oap = out.ap()
    f32 = mybir.dt.float32
    with tile.TileContext(nc) as tc:
        with ExitStack() as ctx:
            xpool = ctx.enter_context(tc.tile_pool(name="x", bufs=3))
            ypool = ctx.enter_context(tc.tile_pool(name="y", bufs=4))
            zpool = ctx.enter_context(tc.tile_pool(name="z", bufs=4))
            opool = ctx.enter_context(tc.tile_pool(name="o", bufs=5))
            vpool = ctx.enter_context(tc.tile_pool(name="v8", bufs=4))
            xt2 = None
            for b in range(NBLOCKS):
                r0 = b * ROWS_PER_BLOCK
                dst = oap[r0:r0 + ROWS_PER_BLOCK, :].rearrange(
                    "(p r) e -> p (r e)", p=128)
                src = xap[r0:r0 + ROWS_PER_BLOCK, :].rearrange(
                    "(p r) e -> p (r e)", p=128)
                split_load = b < 2 or b >= NBLOCKS - 2
                if split_load:
                    if b % 2 == 0:
                        xt2 = xpool.tile([128, 2 * BLOCK_FREE], f32,
                                         tag="x2")
                    nc.gpsimd.dma_start(
                        xt2[:, (b % 2) * BLOCK_FREE:
                            (b % 2 + 1) * BLOCK_FREE], src)
                elif b % 2 == 0:
                    # partition p holds rows [r0+8p : r0+8p+8] of block b
                    # (c=0) then the same rows of block b+1 (c=1)
                    src2 = xap[r0:r0 + 2 * ROWS_PER_BLOCK, :].rearrange(
                        "(c p r) e -> p c (r e)", c=2, p=128)
                    xt2 = xpool.tile([128, 2 * BLOCK_FREE], f32, tag="x2")
                    nc.gpsimd.dma_start(xt2[:], src2)
                boff = (b % 2) * BLOCK_FREE
                yt = ypool.tile([128, BLOCK_FREE], f32)
                v8 = vpool.tile([128, 8 * ROWS_PER_PART], f32)
                zt = zpool.tile([128, BLOCK_FREE], f32)
                for h in range(2):
                    hs = h * SEGS_PER_HALF
                    for s in range(hs, hs + SEGS_PER_HALF):
                        seg = slice(boff + s * E, boff + (s + 1) * E)
                        nc.vector.max(v8[:, s * 8:(s + 1) * 8], xt2[:, seg])
                    for s in range(hs, hs + SEGS_PER_HALF):
                        seg = slice(boff + s * E, boff + (s + 1) * E)
                        nc.vector.match_replace(
                            yt[:, s * E:(s + 1) * E],
                            v8[:, s * 8:(s + 1) * 8], xt2[:, seg], BETA)
                    nc.scalar.activation(
                        zt[:, h * HALF:(h + 1) * HALF],
                        yt[:, h * HALF:(h + 1) * HALF],
                        mybir.ActivationFunctionType.Copy,
                        bias=BETA, scale=-1.0)
                ot = opool.tile([128, BLOCK_FREE], f32)
                nc.gpsimd.tensor_tensor(
                    ot[:], zt[:], xt2[:, boff:boff + BLOCK_FREE],
                    op=mybir.AluOpType.add)
                nc.sync.dma_start(dst, ot[:])
    split_sync_waits(nc)
    return nc


_nc_cache = None


def _get_nc():
    global _nc_cache
    if _nc_cache is None:
        _nc_cache = build()
    return _nc_cache


def kernel(x: np.ndarray, _trace: bool = False, **_trace_kwargs):
    x = np.ascontiguousarray(np.asarray(x, dtype=np.float32))
    assert x.shape == (N, E), x.shape
    nc = _get_nc()
    in_maps = [
        {"x": x[c * ROWS_PER_CORE:(c + 1) * ROWS_PER_CORE]}
        for c in range(NCORES)
    ]
    res = run_bass_kernel_spmd(nc, in_maps, core_ids=list(range(NCORES)),
                               trace=_trace, **_trace_kwargs)
    out = np.concatenate([res.results[c]["out"] for c in range(NCORES)],
                         axis=0)
    if _trace:
        return out, res
    return out
